# revision 1
# baseline (speedup 1.0000x reference)
# Trainium2 Bass kernel for DistNSA (sparse attention, 3 branches).
#
# Strategy: shard the 2048 queries contiguously across 8 NeuronCores (256
# queries each); K/V are replicated to every core.  On each core everything is
# computed in a "transposed" E-layout [keys(part), queries(free)] so the PV
# matmuls need no on-chip transposition of the probability matrices:
#   - window branch: E_win = exp(L) * win01 mask (host-precomputed, position-only)
#   - compressed branch: computed [q(part), blocks(free)] so the softmax
#     denominator falls out of the activation accum and the top-k runs on the
#     free axis (nc.vector.max/max_index reproduce jax.lax.top_k tie-breaks)
#   - selected branch: selection mask expanded blocks->keys via a small matmul
#     and multiplied with a causal host mask; E_slt = exp(L) * selcaus01
# Denominators for the transposed branches come from all-ones-lhsT matmuls
# (which broadcast the per-query sums to all 128 partitions for free);
# normalization uses reciprocal_approx_accurate (~2 ULP).
import numpy as np

import concourse.bass as bass
import concourse.bacc as bacc_mod
import concourse.mybir as mybir
from concourse.tile import TileContext

F32 = mybir.dt.float32
F32R = mybir.dt.float32r
BF16 = mybir.dt.bfloat16
U32 = mybir.dt.uint32
AOT = mybir.ActivationFunctionType
ALU = mybir.AluOpType

S, NHQ, NHK, HD = 2048, 8, 2, 128
REP = NHQ // NHK
WIN, BLK, NB, TOPN = 512, 32, 64, 4
SCALE = float(HD) ** -0.5
NCORE = 8
SQ = S // NCORE          # 256 queries per core
NKT = S // 128           # 16 key tiles
NEG_EPS = 1e-30


def _r(ap):
    return ap.bitcast(F32R)


def build_nc() -> bass.Bass:
    import os
    PHASE = int(os.environ.get("NSA_PHASE", "6"))
    nc = bacc_mod.Bacc("TRN2", target_bir_lowering=False, debug=False)

    # ---------------- DRAM I/O ----------------
    qT_d = nc.dram_tensor("qT", [NHQ, HD, SQ], F32R, kind="ExternalInput")
    qTf_d = nc.dram_tensor("qTf", [NHQ, HD, SQ], F32, kind="ExternalInput")
    kT_d = nc.dram_tensor("kT", [NHK, HD, S], F32R, kind="ExternalInput")
    vb_d = nc.dram_tensor("vb", [NHK, NKT, 128, HD], BF16, kind="ExternalInput")
    bm_d = nc.dram_tensor("bm", [NKT, 128, NB], BF16, kind="ExternalInput")
    win_d = nc.dram_tensor("win01T", [128, NKT * SQ], BF16, kind="ExternalInput")
    caus_d = nc.dram_tensor("caus01T", [128, NKT * SQ], BF16, kind="ExternalInput")
    nval_d = nc.dram_tensor("nvalid", [128, 2], F32, kind="ExternalInput")
    negc_d = nc.dram_tensor("negc", [128, 2, NB], F32, kind="ExternalInput")
    bon_d = nc.dram_tensor("bonus", [128, 2, NB], F32, kind="ExternalInput")
    io64_d = nc.dram_tensor("iota64", [128, NB], F32, kind="ExternalInput")
    grow_d = nc.dram_tensor("grow", [1, 2 * NHQ * SQ], F32, kind="ExternalInput")
    gcq_d = nc.dram_tensor("gcq", [128, 2, NHQ], F32, kind="ExternalInput")
    idf_d = nc.dram_tensor("identf", [128, 128], F32, kind="ExternalInput")
    idb_d = nc.dram_tensor("identb", [128, 128], BF16, kind="ExternalInput")
    on128_d = nc.dram_tensor("ones128", [128, 128], BF16, kind="ExternalInput")
    ex01_d = nc.dram_tensor("expand01", [NB, NKT * 128], BF16, kind="ExternalInput")
    oT_d = nc.dram_tensor("oT", [NHQ, HD, SQ], F32, kind="ExternalOutput")
    dbg_d = nc.dram_tensor("dbg", [NHK, 2, 128, 8], F32, kind="ExternalOutput")

    from contextlib import ExitStack

    with TileContext(nc) as tc, ExitStack() as ctx:
        cpool = ctx.enter_context(tc.tile_pool(name="const", bufs=1))
        wpool = ctx.enter_context(tc.tile_pool(name="work", bufs=2))
        epool = ctx.enter_context(tc.tile_pool(name="espace", bufs=1))
        ps = ctx.enter_context(tc.tile_pool(name="ps", bufs=2, space="PSUM"))
        psL = ctx.enter_context(tc.tile_pool(name="psL", bufs=1, space="PSUM"))
        psPV = ctx.enter_context(tc.tile_pool(name="psPV", bufs=1, space="PSUM"))

        def scratch(shape, dtype=F32, name="scr_ps"):
            return ps.tile(shape, dtype, name=name, tag="ps_scratch")

        # ------------- persistent loads -------------
        qT = cpool.tile([128, NHQ, SQ], F32R, name="qT_s")
        nc.sync.dma_start(qT, qT_d.rearrange("h d q -> d h q"))
        qTf = cpool.tile([128, NHQ, SQ], F32, name="qTf_s")
        nc.sync.dma_start(qTf, qTf_d.rearrange("h d q -> d h q"))
        kT = cpool.tile([128, NHK, S], F32R, name="kT_s")
        nc.sync.dma_start(kT, kT_d.rearrange("g d k -> d g k"))
        vb = cpool.tile([128, NHK, NKT, HD], BF16, name="vb_s")
        nc.sync.dma_start(vb, vb_d.rearrange("g t k d -> k g t d"))
        bm = cpool.tile([128, NKT, NB], BF16, name="bm_s")
        nc.sync.dma_start(bm, bm_d.rearrange("t k n -> k t n"))
        win01 = cpool.tile([128, NKT * SQ], BF16, name="win_s")
        nc.sync.dma_start(win01, win_d[:])
        caus01 = cpool.tile([128, NKT * SQ], BF16, name="caus_s")
        nc.sync.dma_start(caus01, caus_d[:])
        nval = cpool.tile([128, 2], F32, name="nval_s")
        nc.sync.dma_start(nval, nval_d[:])
        negc = cpool.tile([128, 2, NB], F32, name="negc_s")
        nc.sync.dma_start(negc, negc_d[:])
        bon = cpool.tile([128, 2, NB], F32, name="bon_s")
        nc.sync.dma_start(bon, bon_d[:])
        io64 = cpool.tile([128, NB], F32, name="io64_s")
        nc.sync.dma_start(io64, io64_d[:])
        grow = cpool.tile([1, 2 * NHQ * SQ], F32, name="grow_s")
        nc.sync.dma_start(grow, grow_d[:])
        gcq = cpool.tile([128, 2, NHQ], F32, name="gcq_s")
        nc.sync.dma_start(gcq, gcq_d[:])
        idf = cpool.tile([128, 128], F32, name="idf_s")
        nc.sync.dma_start(idf, idf_d[:])
        idb = cpool.tile([128, 128], BF16, name="idb_s")
        nc.sync.dma_start(idb, idb_d[:])
        on128 = cpool.tile([128, 128], BF16, name="on128_s")
        nc.sync.dma_start(on128, on128_d[:])
        ex01 = cpool.tile([NB, NKT * 128], BF16, name="ex01_s")
        nc.sync.dma_start(ex01, ex01_d[:])

        # computed persistents
        kcT = cpool.tile([128, NHK, NB], F32, name="kcT_s")
        vcs = cpool.tile([NB, NHK, HD], BF16, name="vcs_s")
        rcs = cpool.tile([128, NHK, 2, REP], F32, name="rcs_s")

        # block means: kcT via pooled average over kT columns; vc via bm matmul
        for g in range(NHK):
            nc.vector.tensor_reduce(
                out=kcT[:, g],
                in_=kT[:, g].rearrange("p (n b) -> p n b", b=BLK).bitcast(F32),
                axis=mybir.AxisListType.X, op=ALU.add,
            )
            nc.vector.tensor_scalar_mul(kcT[:, g], kcT[:, g], 1.0 / BLK)
            vcp = scratch([NB, HD], name="vcp")
            for kt in range(NKT):
                nc.tensor.matmul(
                    vcp, bm[:, kt], vb[:, g, kt],
                    start=(kt == 0), stop=(kt == NKT - 1),
                )
            nc.vector.tensor_copy(vcs[:, g], vcp)

        for g in range(NHK):
            if PHASE < 2:
                continue
            # ---------------- cmp branch + selection ----------------
            ecT = wpool.tile([NB, REP, SQ], BF16, name="ecT", tag="ecT")
            bTs = wpool.tile([NB, SQ], BF16, name="bTs", tag="bTs")
            for qh in range(2):
                qsl = slice(qh * 128, (qh + 1) * 128)
                pg = [
                    wpool.tile([128, NB], F32, name=f"pg{i}", tag=f"pg{i}")
                    for i in range(2)
                ]
                for r in range(REP):
                    h = g * REP + r
                    lc = scratch([128, NB], name="lc")
                    nc.tensor.matmul(lc, qTf[:, h, qsl], kcT[:, g])
                    lcm = wpool.tile([128, NB], F32, name="lcm", tag="lcm")
                    nc.vector.scalar_tensor_tensor(
                        out=lcm, in0=lc, scalar=SCALE,
                        in1=negc[:, qh], op0=ALU.mult, op1=ALU.add,
                    )
                    ec = wpool.tile([128, NB], F32, name="ec", tag="ec")
                    zc = wpool.tile([128, 1], F32, name="zc", tag="zc")
                    nmx = wpool.tile([128, 1], F32, name="nmx", tag="nmx")
                    nc.vector.tensor_reduce(
                        out=nmx, in_=lcm, axis=mybir.AxisListType.X,
                        op=ALU.max, negate=True,
                    )
                    nc.scalar.activation(ec, lcm, AOT.Exp, bias=nmx, accum_out=zc)
                    nc.vector.tensor_scalar_add(zc, zc, NEG_EPS)
                    nc.vector.reciprocal(rcs[:, g, qh, r:r + 1], zc)
                    if r == 0:
                        nc.vector.tensor_scalar(
                            pg[0], ec, rcs[:, g, qh, r:r + 1], None, op0=ALU.mult
                        )
                    else:
                        nc.vector.scalar_tensor_tensor(
                            out=pg[r % 2], in0=ec, scalar=rcs[:, g, qh, r:r + 1],
                            in1=pg[(r + 1) % 2], op0=ALU.mult, op1=ALU.add,
                        )
                    # stash E_cmp^T (bf16) for the cmp PV
                    ecb = wpool.tile([128, NB], BF16, name="ecb", tag="ecb")
                    nc.vector.tensor_copy(ecb, ec)
                    ectp = scratch([NB, 128], BF16, name="ectp")
                    nc.tensor.transpose(ectp, ecb, idb)
                    nc.vector.tensor_copy(ecT[:, r, qsl], ectp)
                # selection: score = pg + bonus; exact top-4 (ties -> low idx)
                score = wpool.tile([128, NB], F32, name="score", tag="score")
                nc.vector.tensor_add(score, pg[(REP - 1) % 2], bon[:, qh])
                mx8 = wpool.tile([128, 8], F32, name="mx8", tag="mx8")
                nc.vector.max(out=mx8, in_=score)
                ix8 = wpool.tile([128, 8], U32, name="ix8", tag="ix8")
                nc.vector.max_index(ix8, mx8, score)
                ixf = wpool.tile([128, TOPN], F32, name="ixf", tag="ixf")
                nc.vector.tensor_copy(ixf, ix8[:, :TOPN])
                bsel = [
                    wpool.tile([128, NB], BF16, name=f"bsel{i}", tag=f"bsel{i}")
                    for i in range(2)
                ]
                nc.vector.tensor_scalar(
                    bsel[0], io64, ixf[:, 0:1], None, op0=ALU.is_equal
                )
                for t in range(1, TOPN):
                    nc.vector.scalar_tensor_tensor(
                        out=bsel[t % 2], in0=io64, scalar=ixf[:, t:t + 1],
                        in1=bsel[(t + 1) % 2], op0=ALU.is_equal, op1=ALU.add,
                    )
                dbgrow = wpool.tile([128, 8], F32, name="dbgrow", tag="dbgrow")
                nc.vector.tensor_copy(dbgrow, ix8)
                nc.sync.dma_start(dbg_d[g, qh], dbgrow)
                btp = scratch([NB, 128], BF16, name="btp")
                nc.tensor.transpose(btp, bsel[(TOPN - 1) % 2], idb)
                nc.vector.tensor_copy(bTs[:, qsl], btp)

            if PHASE < 3:
                continue
            # ---------------- selection mask expand to keys ----------------
            selc = epool.tile([128, NKT * SQ], BF16, name="selc", tag="selc")
            for kt in range(NKT):
                bex = scratch([128, SQ], name="bex")
                nc.tensor.matmul(
                    bex, ex01[:, kt * 128:(kt + 1) * 128], bTs
                )
                nc.vector.scalar_tensor_tensor(
                    out=selc[:, kt * SQ:(kt + 1) * SQ], in0=bex, scalar=1.0,
                    in1=caus01[:, kt * SQ:(kt + 1) * SQ],
                    op0=ALU.mult, op1=ALU.mult,
                )

            if PHASE < 4:
                continue
            # ---------------- main QK + exp + masks ----------------
            ew = [
                epool.tile([128, NKT * SQ], BF16, name=f"ew{r}", tag=f"ew{r}")
                for r in range(REP)
            ]
            es = [
                epool.tile([128, NKT * SQ], BF16, name=f"es{r}", tag=f"es{r}")
                for r in range(REP)
            ]
            for hp in range(2):          # head pairs
                for grp in range(8):     # groups of 2 key tiles
                    lsp = [
                        psL.tile([128, 2 * SQ], F32, name=f"lsp{hh}")
                        for hh in range(2)
                    ]
                    for kt4 in range(2):
                        kt = grp * 2 + kt4
                        for hh in range(2):
                            r = hp * 2 + hh
                            h = g * REP + r
                            nc.tensor.matmul(
                                lsp[hh][:, kt4 * SQ:(kt4 + 1) * SQ],
                                kT[:, g, kt * 128:(kt + 1) * 128],
                                qT[:, h],
                            )
                    gsl = slice(grp * 2 * SQ, (grp + 1) * 2 * SQ)
                    for hh in range(2):
                        r = hp * 2 + hh
                        esp = wpool.tile([128, 2 * SQ], BF16, name="esp", tag="esp")
                        nc.scalar.activation(esp, lsp[hh], AOT.Exp, scale=SCALE)
                        nc.vector.tensor_mul(ew[r][:, gsl], esp, win01[:, gsl])
                        nc.vector.tensor_mul(es[r][:, gsl], esp, selc[:, gsl])

            if PHASE < 5:
                continue
            # ------- PV + Z + normalize + combine, per head pair -------
            for hp in range(2):
              opvw = [
                  psPV.tile([128, SQ], F32, name=f"opvw{hh}", tag=f"opvw{hh}")
                  for hh in range(2)
              ]
              opvs = [
                  psPV.tile([128, SQ], F32, name=f"opvs{hh}", tag=f"opvs{hh}")
                  for hh in range(2)
              ]
              for kt in range(NKT):
                ksl = slice(kt * SQ, (kt + 1) * SQ)
                for hh in range(2):
                    r = hp * 2 + hh
                    nc.tensor.matmul(
                        opvw[hh], vb[:, g, kt], ew[r][:, ksl],
                        start=(kt == 0), stop=(kt == NKT - 1),
                    )
                    nc.tensor.matmul(
                        opvs[hh], vb[:, g, kt], es[r][:, ksl],
                        start=(kt == 0), stop=(kt == NKT - 1),
                    )
              if PHASE < 6:
                  continue
              for hh in range(2):
                r = hp * 2 + hh
                h = g * REP + r
                acc = wpool.tile([128, SQ], F32, name="acc", tag="acc")
                tmp = wpool.tile([128, SQ], F32, name="tmpc", tag="tmpc")
                for br in range(2):
                    esrc = ew[r] if br == 0 else es[r]
                    zbc = scratch([128, SQ], name="zbc")
                    for kt in range(NKT):
                        nc.tensor.matmul(
                            zbc, on128, esrc[:, kt * SQ:(kt + 1) * SQ],
                            start=(kt == 0), stop=(kt == NKT - 1),
                        )
                    zsb = wpool.tile([128, SQ], F32, name="zsb", tag="zsb")
                    nc.vector.tensor_copy(zsb, zbc)
                    rz = wpool.tile([128, SQ], F32, name="rz", tag="rz")
                    scr = wpool.tile([128, SQ], F32, name="scr", tag="scr")
                    nc.vector.reciprocal_approx_accurate(out=rz, in_=zsb, scratch=scr)
                    # gains broadcast [1,SQ] -> [128,SQ]
                    gb = wpool.tile([128, SQ], F32, name="gb", tag="gb")
                    nc.gpsimd.partition_broadcast(
                        gb, grow[:, (br * NHQ + h) * SQ:(br * NHQ + h + 1) * SQ]
                    )
                    gr = wpool.tile([128, SQ], F32, name="gr", tag="gr")
                    nc.vector.tensor_mul(gr, gb, rz)
                    if br == 0:
                        pvt = wpool.tile([128, SQ], F32, name="pvt", tag="pvt")
                        nc.vector.tensor_mul(pvt, opvw[hh], gr)
                        nc.vector.tensor_copy(acc, pvt)
                    else:
                        nc.vector.tensor_mul(tmp, opvs[hh], gr)
                        nc.vector.tensor_add(acc, acc, tmp)
                # cmp branch: PV + per-partition normalize, then transpose-add
                for qh in range(2):
                    qsl = slice(qh * 128, (qh + 1) * 128)
                    ocp = scratch([128, HD], name="ocp")
                    nc.tensor.matmul(ocp, ecT[:, r, qsl], vcs[:, g])
                    gcr = wpool.tile([128, 1], F32, name="gcr", tag="gcr")
                    nc.vector.tensor_mul(
                        gcr, gcq[:, qh, h:h + 1], rcs[:, g, qh, r:r + 1]
                    )
                    ocs = wpool.tile([128, HD], BF16, name="ocs", tag="ocs")
                    nc.vector.tensor_scalar(ocs, ocp, gcr, None, op0=ALU.mult)
                    octp = scratch([128, 128], BF16, name="octp")
                    nc.tensor.transpose(octp, ocs, idb)
                    nc.vector.tensor_add(acc[:, qsl], acc[:, qsl], octp)
                nc.sync.dma_start(oT_d[h], acc)

        if PHASE < 6:
            for h in range(NHQ):
                accz = wpool.tile([128, SQ], F32, name="accz", tag="accz")
                nc.vector.memset(accz, 0.0)
                nc.sync.dma_start(oT_d[h], accz)

    nc.finalize()
    return nc


# ------------------------- host side -------------------------

def _host_inputs(core: int, q, k, v, g_win, g_cmp, g_slt):
    q_off = core * SQ
    qc = q[q_off:q_off + SQ]                       # [SQ, 8, 128]
    s_glob = np.arange(q_off, q_off + SQ)
    kpos = np.arange(S)

    dif = s_glob[None, :] - kpos[:, None]          # [S(key), SQ(q)]
    win01 = ((dif >= 0) & (dif < WIN)).astype(np.float32)
    caus01 = (dif >= 0).astype(np.float32)
    # [S, SQ] -> [128, NKT*SQ] with column = kt*SQ + jq
    win01T = win01.reshape(NKT, 128, SQ).transpose(1, 0, 2).reshape(128, NKT * SQ)
    caus01T = caus01.reshape(NKT, 128, SQ).transpose(1, 0, 2).reshape(128, NKT * SQ)

    nvalid = np.minimum((s_glob + 1) // BLK, NB).astype(np.float32)
    bonus = np.zeros((SQ, NB), np.float32)
    bonus[np.arange(SQ), np.minimum(s_glob // BLK, NB - 1)] += 1e6
    bonus[:, 0] += 1e6

    bmat = np.zeros((S, NB), np.float32)
    bmat[np.arange(S), kpos // BLK] = 1.0 / BLK
    ex01 = np.zeros((NB, S), np.float32)
    ex01[kpos // BLK, np.arange(S)] = 1.0

    grow = np.stack([g_win[q_off:q_off + SQ].T, g_slt[q_off:q_off + SQ].T])

    qT_host = np.ascontiguousarray(qc.transpose(1, 2, 0))
    return {
        "qT": qT_host,
        "qTf": qT_host,
        "kT": np.ascontiguousarray(k.transpose(1, 2, 0)),
        "vb": np.ascontiguousarray(
            v.transpose(1, 0, 2).reshape(NHK, NKT, 128, HD)
        ).astype(np.float32).astype(_bf16()),
        "bm": bmat.reshape(NKT, 128, NB).astype(_bf16()),
        "win01T": win01T.astype(_bf16()),
        "caus01T": caus01T.astype(_bf16()),
        "nvalid": np.ascontiguousarray(nvalid.reshape(2, 128).T),
        "negc": np.ascontiguousarray(
            np.where(
                np.arange(NB)[None, :] < nvalid[:, None], 0.0, -1e30
            ).astype(np.float32).reshape(2, 128, NB).transpose(1, 0, 2)
        ),
        "bonus": np.ascontiguousarray(
            bonus.reshape(2, 128, NB).transpose(1, 0, 2)
        ),
        "iota64": np.broadcast_to(
            np.arange(NB, dtype=np.float32), (128, NB)
        ).copy(),
        "grow": grow.reshape(1, 2 * NHQ * SQ).astype(np.float32).copy(),
        "gcq": np.ascontiguousarray(
            (g_cmp[q_off:q_off + SQ] * (nvalid > 0)[:, None])
            .reshape(2, 128, NHQ).transpose(1, 0, 2)
        ),
        "identf": np.eye(128, dtype=np.float32),
        "identb": np.eye(128, dtype=np.float32).astype(_bf16()),
        "ones128": np.ones((128, 128), np.float32).astype(_bf16()),
        "expand01": ex01.astype(_bf16()),
    }


def _bf16():
    import ml_dtypes
    return ml_dtypes.bfloat16


_CACHE = {}


def kernel(q, k, v, g_win, g_cmp, g_slt):
    q = np.asarray(q, np.float32)
    k = np.asarray(k, np.float32)
    v = np.asarray(v, np.float32)
    g_win = np.asarray(g_win, np.float32)
    g_cmp = np.asarray(g_cmp, np.float32)
    g_slt = np.asarray(g_slt, np.float32)

    from concourse.bass_utils import run_bass_kernel_spmd

    if "nc" not in _CACHE:
        _CACHE["nc"] = build_nc()
    nc = _CACHE["nc"]

    in_maps = [
        _host_inputs(c, q, k, v, g_win, g_cmp, g_slt) for c in range(NCORE)
    ]
    import os
    res = run_bass_kernel_spmd(
        nc, in_maps, core_ids=list(range(NCORE)),
        trace=bool(int(os.environ.get("NSA_TRACE", "0"))),
    )
    out = np.empty((S, NHQ, HD), np.float32)
    for c in range(NCORE):
        oT = res.results[c]["oT"]                  # [8, 128, 256]
        out[c * SQ:(c + 1) * SQ] = oT.transpose(2, 0, 1)
    _CACHE["last_result"] = res
    return out



# revision 9
# speedup vs baseline: 1.9779x; 1.9779x over previous
# Trainium2 Bass kernel for DistNSA (sparse attention, 3 branches).
#
# Strategy (v2): causal-balanced query sharding with per-core PACKED keys.
#   - Core c owns two 128-query tiles: shallow t0=c and deep t1=8+c.  Every
#     core's key space is re-packed by the host into 16 tiles covering global
#     tiles (c-7..c+8) (zero-padded on the left), so the shallow slot's causal
#     span is always packed tiles 0..7 and the deep slot's is 0..15 with the
#     diagonal at the end.  This keeps the SPMD program uniform while cutting
#     the dense-attention work from 32 to 24 key-tile units per core.
#   - Everything is computed in the transposed E-layout [keys(part),
#     (head, query)(free)], 4 heads of a kv group batched per matmul.
#   - Window branch: only the last 5 packed tiles can intersect the 512-key
#     window; middle tiles are used unmasked (mask==1 there), the band and
#     diagonal (and shallow padding) get host masks.
#   - Selected branch: block-selection one-hots are expanded to keys with a
#     small matmul and multiplied into exp(logits); padded/invalid blocks are
#     never selected so padded tiles contribute exactly zero.
#   - Normalization + gating + branch combine happen on the HOST: the kernel
#     DMAs out unnormalized PV accumulators and Z rows per branch, plus the
#     compressed-branch output and its softmax denominators.
#   - Selection scores (compressed branch) stay in fp32 end-to-end (host-
#     computed fp32 block means, fp32 PE matmul, fp32 softmax) to reproduce
#     jax's top-k tie-breaking exactly; the heavy attention path runs fp16.
import numpy as np

import concourse.bass as bass
import concourse.bacc as bacc_mod
import concourse.mybir as mybir
from concourse.tile import TileContext

F32 = mybir.dt.float32
F32R = mybir.dt.float32r
E16 = mybir.dt.float16
U32 = mybir.dt.uint32
AOT = mybir.ActivationFunctionType
ALU = mybir.AluOpType

S, NHQ, NHK, HD = 2048, 8, 2, 128
REP = NHQ // NHK
WIN, BLK, NB, TOPN = 512, 32, 64, 4
SCALE = float(HD) ** -0.5
NCORE = 8
QT = 128                 # queries per slot
L0, L1 = 8, 16           # packed causal spans (key tiles) per slot
NEG_EPS = 1e-30


def build_nc() -> bass.Bass:
    nc = bacc_mod.Bacc("TRN2", target_bir_lowering=False, debug=False)

    # ---------------- DRAM I/O ----------------
    qTp_d = nc.dram_tensor("qTp", [128, 2, NHQ, QT], E16, kind="ExternalInput")
    qT32_d = nc.dram_tensor("qT32", [128, 2, NHQ, QT], F32, kind="ExternalInput")
    kTp_d = nc.dram_tensor("kTp", [128, NHK, S], E16, kind="ExternalInput")
    vbp_d = nc.dram_tensor("vbp", [128, NHK, 16, HD], E16, kind="ExternalInput")
    kcT_d = nc.dram_tensor("kcT", [128, NHK, NB], F32, kind="ExternalInput")
    vcs_d = nc.dram_tensor("vcs", [NB, NHK, HD], E16, kind="ExternalInput")
    caus_d = nc.dram_tensor("caus4", [128, 512], E16, kind="ExternalInput")
    winm_d = nc.dram_tensor("winm4", [128, 4, 512], E16, kind="ExternalInput")
    winb_d = nc.dram_tensor("winb4", [128, 512], E16, kind="ExternalInput")
    negc_d = nc.dram_tensor("negc8", [128, 2, 512], F32, kind="ExternalInput")
    bon_d = nc.dram_tensor("bonus", [128, 2, NB], F32, kind="ExternalInput")
    io64_d = nc.dram_tensor("iota64", [128, NB], F32, kind="ExternalInput")
    ex01_d = nc.dram_tensor("expand01", [NB, S], E16, kind="ExternalInput")
    idb_d = nc.dram_tensor("identh", [128, 128], E16, kind="ExternalInput")
    on128_d = nc.dram_tensor("ones128", [128, 128], E16, kind="ExternalInput")

    opvw_d = nc.dram_tensor("opvw", [2, NHK, 128, 512], E16, kind="ExternalOutput")
    opvs_d = nc.dram_tensor("opvs", [2, NHK, 128, 512], E16, kind="ExternalOutput")
    zr_d = nc.dram_tensor("zrows", [2, NHK, 2, 512], F32, kind="ExternalOutput")
    ocp_d = nc.dram_tensor("ocp", [2, NHK, 128, 512], E16, kind="ExternalOutput")
    rc_d = nc.dram_tensor("rc", [128, 2, NHQ], F32, kind="ExternalOutput")

    from contextlib import ExitStack

    with TileContext(nc) as tc, ExitStack() as ctx:
        cpool = ctx.enter_context(tc.tile_pool(name="const", bufs=1))
        apool = ctx.enter_context(tc.tile_pool(name="aphase", bufs=2))
        epool = ctx.enter_context(tc.tile_pool(name="espace", bufs=2))
        psQK = ctx.enter_context(tc.tile_pool(name="psQK", bufs=2, space="PSUM"))
        psSel = ctx.enter_context(tc.tile_pool(name="psSel", bufs=2, space="PSUM"))
        psPV = ctx.enter_context(tc.tile_pool(name="psPV", bufs=1, space="PSUM"))

        # ------------- persistent loads (A-phase needs first) -------------
        qT32 = cpool.tile([128, 2, NHQ, QT], F32, name="qT32_s")
        nc.sync.dma_start(qT32, qT32_d[:])
        kcT = cpool.tile([128, NHK, NB], F32, name="kcT_s")
        nc.sync.dma_start(kcT, kcT_d[:])
        negc8 = cpool.tile([128, 2, 512], F32, name="negc8_s")
        nc.sync.dma_start(negc8, negc_d[:])
        bon = cpool.tile([128, 2, NB], F32, name="bon_s")
        nc.sync.dma_start(bon, bon_d[:])
        io64 = cpool.tile([128, NB], F32, name="io64_s")
        nc.sync.dma_start(io64, io64_d[:])
        idb = cpool.tile([128, 128], E16, name="idb_s")
        nc.sync.dma_start(idb, idb_d[:])
        vcs = cpool.tile([NB, NHK, HD], E16, name="vcs_s")
        nc.sync.dma_start(vcs, vcs_d[:])
        qTp = cpool.tile([128, 2, NHQ, QT], E16, name="qTp_s")
        nc.sync.dma_start(qTp, qTp_d[:])
        kTp = cpool.tile([128, NHK, S], E16, name="kTp_s")
        nc.sync.dma_start(kTp, kTp_d[:])
        ex01 = cpool.tile([NB, S], E16, name="ex01_s")
        nc.sync.dma_start(ex01, ex01_d[:])
        vbp = cpool.tile([128, NHK, 16, HD], E16, name="vbp_s")
        nc.sync.dma_start(vbp, vbp_d[:])
        caus4 = cpool.tile([128, 512], E16, name="caus4_s")
        nc.sync.dma_start(caus4, caus_d[:])
        winm4 = cpool.tile([128, 4, 512], E16, name="winm4_s")
        nc.sync.dma_start(winm4, winm_d[:])
        winb4 = cpool.tile([128, 512], E16, name="winb4_s")
        nc.sync.dma_start(winb4, winb_d[:])
        on128 = cpool.tile([128, 128], E16, name="on128_s")
        nc.sync.dma_start(on128, on128_d[:])

        bTs4 = cpool.tile([NB, 2, NHK, REP, QT], E16, name="bTs4_s")
        rc8 = cpool.tile([128, 2, NHQ], F32, name="rc8_s")

        # ---------------- phase A: cmp branch + selection ----------------
        def phase_a(j):
            lc8 = psQK.tile([128, NHQ, NB], F32, name="lc8", tag="qk")
            for g in range(NHK):
                for r in range(REP):
                    h = g * REP + r
                    nc.tensor.matmul(lc8[:, h], qT32[:, j, h], kcT[:, g])
            lcm8 = apool.tile([128, NHQ, NB], F32, name="lcm8", tag="lcm8")
            nc.vector.scalar_tensor_tensor(
                out=lcm8, in0=lc8, scalar=SCALE,
                in1=negc8[:, j].rearrange("p (h n) -> p h n", n=NB),
                op0=ALU.mult, op1=ALU.add,
            )
            ec8 = apool.tile([128, NHQ, NB], F32, name="ec8", tag="ec8")
            nc.scalar.activation(ec8, lcm8, AOT.Exp)
            zc8 = apool.tile([128, NHQ], F32, name="zc8", tag="zc8")
            nc.vector.tensor_reduce(
                out=zc8, in_=ec8, axis=mybir.AxisListType.X, op=ALU.add,
            )
            nc.vector.tensor_scalar_add(zc8, zc8, NEG_EPS)
            nc.vector.reciprocal(rc8[:, j], zc8)
            ecb8 = apool.tile([128, NHQ, NB], E16, name="ecb8", tag="ecb8")
            nc.vector.tensor_copy(ecb8, ec8)

            for g in range(NHK):
                pg = [
                    apool.tile([128, NB], F32, name=f"pg{i}", tag=f"pg{i}")
                    for i in range(2)
                ]
                for r in range(REP):
                    h = g * REP + r
                    if r == 0:
                        nc.vector.tensor_scalar(
                            pg[0], ec8[:, h], rc8[:, j, h:h + 1], None,
                            op0=ALU.mult,
                        )
                    else:
                        nc.vector.scalar_tensor_tensor(
                            out=pg[r % 2], in0=ec8[:, h],
                            scalar=rc8[:, j, h:h + 1],
                            in1=pg[(r + 1) % 2], op0=ALU.mult, op1=ALU.add,
                        )
                score = apool.tile([128, NB], F32, name="score", tag="score")
                nc.vector.tensor_add(score, pg[(REP - 1) % 2], bon[:, j])
                mx8 = apool.tile([128, 8], F32, name="mx8", tag="mx8")
                nc.vector.max(out=mx8, in_=score)
                ix8 = apool.tile([128, 8], U32, name="ix8", tag="ix8")
                nc.vector.max_index(ix8, mx8, score)
                ixf = apool.tile([128, TOPN], F32, name="ixf", tag="ixf")
                nc.vector.tensor_copy(ixf, ix8[:, :TOPN])
                bsel = [
                    apool.tile([128, NB], E16, name=f"bsel{i}", tag=f"bsel{i}")
                    for i in range(2)
                ]
                nc.vector.tensor_scalar(
                    bsel[0], io64, ixf[:, 0:1], None, op0=ALU.is_equal
                )
                for t in range(1, TOPN):
                    nc.vector.scalar_tensor_tensor(
                        out=bsel[t % 2], in0=io64, scalar=ixf[:, t:t + 1],
                        in1=bsel[(t + 1) % 2], op0=ALU.is_equal, op1=ALU.add,
                    )
                btp = psQK.tile([NB, 128], E16, name="btp", tag="qk")
                nc.tensor.transpose(btp, bsel[(TOPN - 1) % 2], idb)
                nc.vector.tensor_copy(
                    bTs4[:, j, g],
                    btp[:, None, :].broadcast_to([NB, REP, QT]),
                )
                # cmp PV: transpose ec per head, PV against block-mean V
                ocp4 = psQK.tile([128, REP, HD], F32, name="ocp4", tag="qk")
                for r in range(REP):
                    h = g * REP + r
                    ectp = psSel.tile([NB, 128], E16, name="ectp", tag="sel")
                    nc.tensor.transpose(ectp, ecb8[:, h], idb)
                    ecT = apool.tile([NB, 128], E16, name="ecT", tag="ecT")
                    nc.scalar.activation(ecT, ectp, AOT.Copy)
                    nc.tensor.matmul(ocp4[:, r], ecT, vcs[:, g])
                oc16 = apool.tile([128, REP, HD], E16, name="oc16", tag="oc16")
                nc.scalar.activation(oc16, ocp4, AOT.Copy)
                nc.sync.dma_start(ocp_d[j, g], oc16)

        # ---------------- phase B: QK + exp + masks ----------------
        # returns (espb, esb, eww) for phase C
        def phase_b(j, g):
            L = L1 if j else L0
            espb = epool.tile([128, 16, 512], E16, name="espb", tag="espb")
            esb = epool.tile([128, 16, 512], E16, name="esb", tag="esb")
            eww = {}
            sel_diag = None
            for kt in range(L):
                qk = psQK.tile([128, 512], F32, name="qk", tag="qk")
                nc.tensor.matmul(
                    qk, kTp[:, g, kt * 128:(kt + 1) * 128],
                    qTp[:, j, g * REP:(g + 1) * REP],
                )
                nc.scalar.activation(espb[:, kt], qk, AOT.Exp, scale=SCALE)
                sel = psSel.tile([128, 512], F32, name="sel", tag="sel")
                nc.tensor.matmul(
                    sel, ex01[:, kt * 128:(kt + 1) * 128], bTs4[:, j, g]
                )
                if kt == L - 1:
                    sel_diag = sel
                elif kt % 2 == 0:
                    s16 = epool.tile([128, 512], E16, name="s16", tag="s16")
                    nc.scalar.activation(s16, sel, AOT.Copy)
                    nc.vector.tensor_mul(esb[:, kt], espb[:, kt], s16)
                else:
                    nc.vector.tensor_mul(esb[:, kt], espb[:, kt], sel)
            # window-branch masked tiles
            if j == 0:
                ewm = epool.tile([128, 4, 512], E16, name="ewm", tag="ewm")
                for i in range(4):
                    nc.vector.tensor_mul(
                        ewm[:, i], espb[:, 3 + i], winm4[:, i]
                    )
                eww = {3: ewm[:, 0], 4: ewm[:, 1], 5: ewm[:, 2], 6: ewm[:, 3]}
            else:
                ewb = epool.tile([128, 512], E16, name="ewb", tag="ewb")
                nc.vector.tensor_mul(ewb, espb[:, L1 - 5], winb4)
                eww = {L1 - 5: ewb}
            ewd = epool.tile([128, 512], E16, name="ewd", tag="ewd")
            nc.vector.tensor_mul(ewd, espb[:, L - 1], caus4)
            eww[L - 1] = ewd
            nc.vector.tensor_mul(esb[:, L - 1], ewd, sel_diag)
            return espb, esb, eww

        # ---------------- phase C: PV + Z accumulation + DMA out ----------
        def phase_c(j, g, espb, esb, eww):
            L = L1 if j else L0
            w0 = L - 5 if j else 3
            opvw = psPV.tile([128, 512], F32, name="opvw", tag="opvw")
            opvs = psPV.tile([128, 512], F32, name="opvs", tag="opvs")
            zw = psPV.tile([128, 512], F32, name="zw", tag="zw")
            zs = psPV.tile([128, 512], F32, name="zs", tag="zs")
            for kt in range(L):
                st, sp = (kt == 0), (kt == L - 1)
                nc.tensor.matmul(opvs, vbp[:, g, kt], esb[:, kt],
                                 start=st, stop=sp)
                nc.tensor.matmul(zs, on128, esb[:, kt], start=st, stop=sp)
                if kt >= w0:
                    rhs_w = eww.get(kt)
                    if rhs_w is None:
                        rhs_w = espb[:, kt]
                    stw, spw = (kt == w0), (kt == L - 1)
                    nc.tensor.matmul(opvw, vbp[:, g, kt], rhs_w,
                                     start=stw, stop=spw)
                    nc.tensor.matmul(zw, on128, rhs_w, start=stw, stop=spw)
            ow16 = epool.tile([128, 512], E16, name="ow16", tag="ow16")
            nc.scalar.activation(ow16, opvw, AOT.Copy)
            nc.sync.dma_start(opvw_d[j, g], ow16)
            os16 = epool.tile([128, 512], E16, name="os16", tag="os16")
            nc.vector.tensor_copy(os16, opvs)
            nc.sync.dma_start(opvs_d[j, g], os16)
            zrw = epool.tile([1, 512], F32, name="zrw", tag="zrw")
            nc.scalar.activation(zrw, zw[0:1, :], AOT.Copy)
            nc.sync.dma_start(zr_d[j, g, 0:1], zrw)
            zrs = epool.tile([1, 512], F32, name="zrs", tag="zrs")
            nc.vector.tensor_copy(zrs, zs[0:1, :])
            nc.sync.dma_start(zr_d[j, g, 1:2], zrs)

        # ---------------- emission schedule ----------------
        phase_a(1)
        phase_a(0)
        units = [(1, 0), (1, 1), (0, 0), (0, 1)]
        pending = []
        for i, (j, g) in enumerate(units):
            st = phase_b(j, g)
            pending.append(((j, g), st))
            if i >= 1:
                (pj, pg_), pst = pending.pop(0)
                phase_c(pj, pg_, *pst)
        for (pj, pg_), pst in pending:
            phase_c(pj, pg_, *pst)
        nc.sync.dma_start(rc_d[:], rc8)

    nc.finalize()
    return nc


# ------------------------- host side -------------------------

def _f16():
    return np.float16


def _host_inputs(core: int, q, k, v):
    c = core
    pad = 7 - c              # packed tile p <-> global tile p - pad
    t0, t1 = c, 8 + c

    kp = np.zeros((S, NHK, HD), np.float32)
    vp = np.zeros((S, NHK, HD), np.float32)
    kp[pad * 128:] = k[:(16 - pad) * 128]
    vp[pad * 128:] = v[:(16 - pad) * 128]

    qq = np.stack([q[t0 * 128:(t0 + 1) * 128], q[t1 * 128:(t1 + 1) * 128]])
    qT = np.ascontiguousarray(qq.transpose(3, 0, 2, 1))   # [128,2,8,128]

    jq = np.arange(QT)
    p = np.arange(128)
    caus = (jq[None, :] >= p[:, None]).astype(np.float32)        # [128,128]
    band = (jq[None, :] < p[:, None]).astype(np.float32)
    caus4 = np.tile(caus, (1, REP))
    winb4 = np.tile(band, (1, REP))
    winm = np.zeros((128, 4, 512), np.float32)
    for i in range(4):
        kt = 3 + i
        if kt - pad < 0:
            continue
        m = band if i == 0 else np.ones((128, 128), np.float32)
        winm[:, i] = np.tile(m, (1, REP))

    # valid packed blocks per (slot, query-row)
    negc8 = np.zeros((128, 2, 512), np.float32)
    bon = np.zeros((128, 2, NB), np.float32)
    b = np.arange(NB)
    for j, t in ((0, t0), (1, t1)):
        nval = (t * 128 + jq + 1) // BLK                  # global blocks valid
        gb = b[None, :] - 4 * pad                          # global block id
        valid = (gb >= 0) & (gb < nval[:, None])
        negc = np.where(valid, 0.0, -1e30).astype(np.float32)    # [128,64]
        negc8[:, j] = np.tile(negc, (1, NHQ))
        bon[jq, j, 4 * pad] += 1e6
        bon[jq, j, 4 * (t + 7 - c) + jq // BLK] += 1e6

    pk = np.arange(S)
    ex01 = ((pk[None, :] // BLK == b[:, None])
            & (b[:, None] >= 4 * pad)).astype(np.float32)

    kcT = kp.reshape(NB, BLK, NHK, HD).mean(1)             # [64,2,128] f32
    vcs = vp.reshape(NB, BLK, NHK, HD).mean(1)

    return {
        "qTp": qT.astype(_f16()),
        "qT32": qT,
        "kTp": np.ascontiguousarray(kp.transpose(2, 1, 0)).astype(_f16()),
        "vbp": np.ascontiguousarray(
            vp.reshape(16, 128, NHK, HD).transpose(1, 2, 0, 3)
        ).astype(_f16()),
        "kcT": np.ascontiguousarray(kcT.transpose(2, 1, 0)),
        "vcs": np.ascontiguousarray(vcs).astype(_f16()),
        "caus4": caus4.astype(_f16()),
        "winm4": winm.astype(_f16()),
        "winb4": winb4.astype(_f16()),
        "negc8": negc8,
        "bonus": bon,
        "iota64": np.broadcast_to(b.astype(np.float32), (128, NB)).copy(),
        "expand01": ex01.astype(_f16()),
        "identh": np.eye(128, dtype=np.float32).astype(_f16()),
        "ones128": np.ones((128, 128), np.float32).astype(_f16()),
    }


_CACHE = {}


def kernel(q, k, v, g_win, g_cmp, g_slt):
    q = np.asarray(q, np.float32)
    k = np.asarray(k, np.float32)
    v = np.asarray(v, np.float32)
    g_win = np.asarray(g_win, np.float32)
    g_cmp = np.asarray(g_cmp, np.float32)
    g_slt = np.asarray(g_slt, np.float32)

    from concourse.bass_utils import run_bass_kernel_spmd

    if "nc" not in _CACHE:
        _CACHE["nc"] = build_nc()
    nc = _CACHE["nc"]

    in_maps = [_host_inputs(c, q, k, v) for c in range(NCORE)]
    import os
    res = run_bass_kernel_spmd(
        nc, in_maps, core_ids=list(range(NCORE)),
        trace=bool(int(os.environ.get("NSA_TRACE", "0"))),
    )
    _CACHE["last_result"] = res

    out = np.empty((S, NHQ, HD), np.float32)
    for c in range(NCORE):
        r = res.results[c]
        opvw = r["opvw"].astype(np.float32)
        opvs = r["opvs"].astype(np.float32)
        zr = r["zrows"]
        ocp, rc = r["ocp"].astype(np.float32), r["rc"]
        for j, t in ((0, c), (1, 8 + c)):
            qs = slice(t * 128, (t + 1) * 128)
            for g in range(NHK):
                # [128d, 4, 128jq] -> [jq, r, d]
                ow = opvw[j, g].reshape(HD, REP, QT) / zr[j, g, 0].reshape(REP, QT)
                os_ = opvs[j, g].reshape(HD, REP, QT) / zr[j, g, 1].reshape(REP, QT)
                oc = ocp[j, g].reshape(QT, REP, HD) * rc[:, j, g * REP:(g + 1) * REP][:, :, None]
                hs = slice(g * REP, (g + 1) * REP)
                out[qs, hs] = (
                    g_win[qs, hs, None] * ow.transpose(2, 1, 0)
                    + g_slt[qs, hs, None] * os_.transpose(2, 1, 0)
                    + g_cmp[qs, hs, None] * oc
                )
    return out
